# revision 1
# baseline (speedup 1.0000x reference)
"""Grouped per-sample MLP (conv1d groups=B) + GroupSwish + softmax, on 8 NeuronCores.

Data-parallel over the group/batch axis B=256: 32 groups per core.
Per group g: h = W1[g] @ x[g] + b1[g]; GroupSwish; o = W2[g] @ h + b2[g];
softmax over the flattened [C*L] logits.

Device strategy per core (per group, fully unrolled):
  - W1 matmul out[32, 512], contraction X=784 split 6x128 + 16, operands fed
    as float32r (TF32-like, 1 PE cycle/row, HW rounds internally) straight
    from DMA. fp32r matmuls must write PSUM at partition base 0.
  - GroupSwish via tanh (the only ACT table with both tanh and exp):
    (h+b1)*sigmoid(sp*(h+b1)) = ((h+b1)*0.5) * (1 + tanh(sp*(h+b1)/2)).
    The 1/1.1 factor is folded into W2 host-side; sp = softplus(beta) is
    computed on device via exp/ln.
  - Softmax without max-subtraction (logits are O(1)): exp with fused
    per-partition accum, cross-partition sum / broadcast via tiny matmuls
    against ones vectors.
"""

import os
import numpy as np
from contextlib import ExitStack

import concourse.mybir as mybir
import concourse.tile as tile
from concourse import bacc
from concourse.bass_utils import run_bass_kernel_spmd

B, X, Z, C, L = 256, 784, 32, 10, 512
NCORE = 8
GPC = B // NCORE  # 32 groups per core
NCH = 7  # K-chunks: 6*128 + 16
KLAST = X - 6 * 128  # 16
P = 128
F32 = mybir.dt.float32
F32R = mybir.dt.float32r

DEFAULT_CFG = dict(
    x_layout="interleave",  # "interleave": chunk c = rows 128c+p, 2KB runs;
    #                         "contig": one run/partition (uneven 7/6 rows)
    x_engines=("sync",),  # trigger engines for x loads, round-robin by group
    w_engine="sync",
    out_engine="gpsimd",
    const_engine="gpsimd",
    x_bufs=6,
    h_bufs=3,
    s_bufs=3,
    x_split=False,  # split each group's x-main DMA across sync+scalar queues
    x_pair=False,  # load two groups' x per DMA (halves trigger count)
    pipeline=False,  # defer W2 by one quad and softmax-normalize per quad,
    #                  two quads behind, to keep the PE stream stall-free
)

_CACHE: dict = {}


def _eng(nc, name):
    return getattr(nc, name)


def _build(cfg=DEFAULT_CFG):
    if cfg.get("pipeline"):
        return _build_pipelined(cfg)
    nc = bacc.Bacc("TRN2", target_bir_lowering=False, debug=False)

    xg = nc.dram_tensor("xg", [GPC, X, L], F32R, kind="ExternalInput").ap()
    # W1T packed per quad of groups; each partition reads one contiguous
    # 4*7*32*4B run. w1m[gq, p, j, c, z] = W1[4gq+j][z, row(p, c)] where
    # row depends on x_layout (see _marshal).
    w1m = nc.dram_tensor(
        "w1m", [GPC // 4, P, 4, NCH, Z], F32R, kind="ExternalInput"
    ).ap()
    w2t = nc.dram_tensor("w2t", [Z, GPC * C], F32R, kind="ExternalInput").ap()
    b1c = nc.dram_tensor("b1c", [Z, GPC], F32, kind="ExternalInput").ap()
    btc = nc.dram_tensor("btc", [Z, GPC], F32, kind="ExternalInput").ap()
    b2c = nc.dram_tensor("b2c", [C, GPC], F32, kind="ExternalInput").ap()
    out = nc.dram_tensor("out", [GPC, C, L], F32, kind="ExternalOutput").ap()

    with tile.TileContext(nc) as tc, ExitStack() as ctx:
        consts = ctx.enter_context(tc.tile_pool(name="consts", bufs=1))
        xpool = ctx.enter_context(tc.tile_pool(name="x", bufs=cfg["x_bufs"]))
        wpool = ctx.enter_context(tc.tile_pool(name="w1", bufs=3))
        spool = ctx.enter_context(tc.tile_pool(name="act", bufs=cfg["s_bufs"]))
        hps = ctx.enter_context(
            tc.tile_pool(name="hps", bufs=cfg["h_bufs"], space="PSUM")
        )
        ops = ctx.enter_context(tc.tile_pool(name="ops", bufs=2, space="PSUM"))
        tps = ctx.enter_context(tc.tile_pool(name="tps", bufs=2, space="PSUM"))

        ce = _eng(nc, cfg["const_engine"])
        we = _eng(nc, cfg["w_engine"])
        oe = _eng(nc, cfg["out_engine"])

        # --- constants / per-group scalars ---
        w2tt = consts.tile([Z, GPC * C], F32R, name="w2tt")
        ce.dma_start(w2tt[:], w2t)
        b1t = consts.tile([Z, GPC], F32, name="b1t")
        ce.dma_start(b1t[:], b1c)
        btt = consts.tile([Z, GPC], F32, name="btt")
        ce.dma_start(btt[:], btc)
        b2t = consts.tile([C, GPC], F32, name="b2t")
        ce.dma_start(b2t[:], b2c)
        ones_k = consts.tile([C, 1], F32, name="ones_k")
        nc.vector.memset(ones_k[:], 1.0)
        ones_m = consts.tile([1, C], F32, name="ones_m")
        nc.vector.memset(ones_m[:], 1.0)

        # sp = softplus(beta) = ln(1 + exp(beta)); halves for tanh-sigmoid
        spe = consts.tile([Z, GPC], F32, name="spe")
        nc.scalar.activation(spe[:], btt[:], mybir.ActivationFunctionType.Exp)
        spe1 = consts.tile([Z, GPC], F32, name="spe1")
        nc.vector.tensor_scalar_add(spe1[:], spe[:], 1.0)
        spt = consts.tile([Z, GPC], F32, name="spt")
        nc.scalar.activation(spt[:], spe1[:], mybir.ActivationFunctionType.Ln)
        sph = consts.tile([Z, GPC], F32, name="sph")
        nc.vector.tensor_scalar_mul(sph[:], spt[:], 0.5)
        spb1h = consts.tile([Z, GPC], F32, name="spb1h")
        nc.vector.tensor_mul(spb1h[:], sph[:], b1t[:])

        xt2 = None
        for g in range(GPC):
            gq, jq = divmod(g, 4)
            xe = _eng(nc, cfg["x_engines"][g % len(cfg["x_engines"])])
            if cfg["x_pair"]:
                # one [P, 2*7*L] tile per pair of groups; group g%2==i owns
                # free columns [i*NCH*L, (i+1)*NCH*L) logically remapped below
                if g % 2 == 0:
                    xt2 = xpool.tile([P, 2 * NCH * L], F32R, tag="xt", name=f"xt{g}")
                    xe.dma_start(
                        xt2[:, : 12 * L].rearrange("p (i c l) -> p i c l", i=2, c=6),
                        xg[g : g + 2, : 6 * P].rearrange("i (c p) l -> p i c l", p=P),
                    )
                    xe.dma_start(
                        xt2[:KLAST, 12 * L :].rearrange("p (i l) -> p i l", i=2),
                        xg[g : g + 2, 6 * P :].rearrange("i r l -> r i l"),
                    )
                i = g % 2
                xt = xt2[:, i * 6 * L : (i + 1) * 6 * L]
                xlast = xt2[:, (12 + i) * L : (13 + i) * L]
            elif cfg["x_layout"] == "interleave":
                # chunk c = rows 128c..128c+128; 2KB runs across partitions
                xt = xpool.tile([P, NCH * L], F32R, tag="xt", name=f"xt{g}")
                xlast = xt[:, 6 * L :]
                if cfg["x_split"]:
                    nc.sync.dma_start(
                        xt[:, : 3 * L].rearrange("p (c l) -> p c l", c=3),
                        xg[g, : 3 * P].rearrange("(c p) l -> p c l", p=P),
                    )
                    nc.scalar.dma_start(
                        xt[:, 3 * L : 6 * L].rearrange("p (c l) -> p c l", c=3),
                        xg[g, 3 * P : 6 * P].rearrange("(c p) l -> p c l", p=P),
                    )
                else:
                    xe.dma_start(
                        xt[:, : 6 * L].rearrange("p (c l) -> p c l", c=6),
                        xg[g, : 6 * P].rearrange("(c p) l -> p c l", p=P),
                    )
                xe.dma_start(xt[:KLAST, 6 * L :], xg[g, 6 * P :])
            else:
                # one contiguous run per partition: p<16 -> rows 7p..7p+7,
                # p>=16 -> rows 112+6(p-16)..+6
                xt = xpool.tile([P, NCH * L], F32R, tag="xt", name=f"xt{g}")
                xlast = xt[:, 6 * L :]
                xe.dma_start(
                    xt[:16, :].rearrange("p (c l) -> p c l", c=NCH),
                    xg[g, : 7 * 16].rearrange("(p c) l -> p c l", p=16),
                )
                xe.dma_start(
                    xt[16:, : 6 * L].rearrange("p (c l) -> p c l", c=6),
                    xg[g, 7 * 16 : X].rearrange("(p c) l -> p c l", p=112),
                )
            # --- W1T for a quad of 4 groups, one DMA every 4th group ---
            if jq == 0:
                wt = wpool.tile([P, 4 * NCH * Z], F32R, tag="wt", name=f"wt{g}")
                we.dma_start(
                    wt[:].rearrange("p (j c z) -> p j c z", j=4, c=NCH),
                    w1m[gq],
                )

            # --- h = W1 @ x ---
            h = hps.tile([Z, L], F32, tag="h", name=f"h{g}")
            for c in range(NCH):
                kk = P if c < 6 else KLAST
                rhs = (
                    xt[:, c * L : (c + 1) * L] if c < 6 else xlast[:KLAST, :]
                )
                nc.tensor.matmul(
                    h[:],
                    wt[:kk, (jq * NCH + c) * Z : (jq * NCH + c + 1) * Z],
                    rhs,
                    start=(c == 0),
                    stop=(c == NCH - 1),
                )

            # --- GroupSwish: ((h+b1)*0.5) * (1 + tanh(sp*(h+b1)/2)) ---
            t = spool.tile([Z, L], F32, tag="t", name=f"t{g}")
            nc.scalar.activation(
                t[:],
                h[:],
                mybir.ActivationFunctionType.Tanh,
                bias=spb1h[:, g : g + 1],
                scale=sph[:, g : g + 1],
            )
            u = spool.tile([Z, L], F32, tag="u", name=f"u{g}")
            nc.vector.tensor_scalar(
                u[:],
                h[:],
                b1t[:, g : g + 1],
                0.5,
                op0=mybir.AluOpType.add,
                op1=mybir.AluOpType.mult,
            )
            swish = spool.tile([Z, L], F32R, tag="swish", name=f"sw{g}")
            nc.vector.scalar_tensor_tensor(
                swish[:],
                t[:],
                1.0,
                u[:],
                op0=mybir.AluOpType.add,
                op1=mybir.AluOpType.mult,
            )

            # --- o = (W2/1.1) @ swish ---
            o = ops.tile([C, L], F32, tag="o", name=f"o{g}")
            nc.tensor.matmul(
                o[:], w2tt[:, g * C : (g + 1) * C], swish[:], start=True, stop=True
            )

            # --- softmax over [C, L] (no max subtraction) ---
            expo = spool.tile([C, L], F32, tag="expo", name=f"e{g}")
            esum = spool.tile([C, 1], F32, tag="esum", name=f"es{g}")
            nc.scalar.activation(
                expo[:],
                o[:],
                mybir.ActivationFunctionType.Exp,
                bias=b2t[:, g : g + 1],
                scale=1.0,
                accum_out=esum[:],
            )
            tot = tps.tile([1, 1], F32, tag="tb", name=f"tot{g}")
            nc.tensor.matmul(tot[:], ones_k[:], esum[:], start=True, stop=True)
            inv = spool.tile([1, 1], F32, tag="inv", name=f"inv{g}")
            nc.vector.reciprocal(inv[:], tot[:])
            bc = tps.tile([C, 1], F32, tag="tb", name=f"bc{g}")
            nc.tensor.matmul(bc[:], ones_m[:], inv[:], start=True, stop=True)
            invc = spool.tile([C, 1], F32, tag="invc", name=f"ic{g}")
            nc.vector.tensor_copy(invc[:], bc[:])
            res = spool.tile([C, L], F32, tag="res", name=f"r{g}")
            nc.vector.tensor_scalar_mul(res[:], expo[:], invc[:])

            oe.dma_start(out[g], res[:])

    nc.compile()
    return nc


def _build_pipelined(cfg):
    """Software-pipelined emission: the PE stream per quad q is
    [28x W1(q)] [4x W2(q-1)] [tot4(q-2), bc4(q-2)] so every cross-engine
    dependency (swish from DVE, exp sums from ACT, reciprocal from DVE) has
    a full quad of slack before the PE needs it."""
    nc = bacc.Bacc("TRN2", target_bir_lowering=False, debug=False)
    NQ = GPC // 4

    xg = nc.dram_tensor("xg", [GPC, X, L], F32R, kind="ExternalInput").ap()
    w1m = nc.dram_tensor(
        "w1m", [NQ, P, 4, NCH, Z], F32R, kind="ExternalInput"
    ).ap()
    w2t = nc.dram_tensor("w2t", [Z, GPC * C], F32R, kind="ExternalInput").ap()
    b1c = nc.dram_tensor("b1c", [Z, GPC], F32, kind="ExternalInput").ap()
    btc = nc.dram_tensor("btc", [Z, GPC], F32, kind="ExternalInput").ap()
    b2c = nc.dram_tensor("b2c", [C, GPC], F32, kind="ExternalInput").ap()
    out = nc.dram_tensor("out", [GPC, C, L], F32, kind="ExternalOutput").ap()

    with tile.TileContext(nc) as tc, ExitStack() as ctx:
        consts = ctx.enter_context(tc.tile_pool(name="consts", bufs=1))
        xpool = ctx.enter_context(tc.tile_pool(name="x", bufs=cfg["x_bufs"]))
        wpool = ctx.enter_context(tc.tile_pool(name="w1", bufs=3))
        spool = ctx.enter_context(tc.tile_pool(name="act", bufs=cfg["s_bufs"]))
        dpool = ctx.enter_context(tc.tile_pool(name="deep", bufs=10))
        e4pool = ctx.enter_context(tc.tile_pool(name="e4", bufs=3))
        hps = ctx.enter_context(
            tc.tile_pool(name="hps", bufs=cfg["h_bufs"], space="PSUM")
        )
        ops = ctx.enter_context(tc.tile_pool(name="ops", bufs=2, space="PSUM"))
        tps = ctx.enter_context(tc.tile_pool(name="tps", bufs=2, space="PSUM"))

        oe = _eng(nc, cfg["out_engine"])
        ce = _eng(nc, cfg["const_engine"])
        we = _eng(nc, cfg["w_engine"])

        w2tt = consts.tile([Z, GPC * C], F32R, name="w2tt")
        ce.dma_start(w2tt[:], w2t)
        b1t = consts.tile([Z, GPC], F32, name="b1t")
        ce.dma_start(b1t[:], b1c)
        btt = consts.tile([Z, GPC], F32, name="btt")
        ce.dma_start(btt[:], btc)
        b2t = consts.tile([C, GPC], F32, name="b2t")
        ce.dma_start(b2t[:], b2c)
        ones_k = consts.tile([C, 1], F32, name="ones_k")
        nc.vector.memset(ones_k[:], 1.0)
        ones_m = consts.tile([1, C], F32, name="ones_m")
        nc.vector.memset(ones_m[:], 1.0)

        spe = consts.tile([Z, GPC], F32, name="spe")
        nc.scalar.activation(spe[:], btt[:], mybir.ActivationFunctionType.Exp)
        spe1 = consts.tile([Z, GPC], F32, name="spe1")
        nc.vector.tensor_scalar_add(spe1[:], spe[:], 1.0)
        spt = consts.tile([Z, GPC], F32, name="spt")
        nc.scalar.activation(spt[:], spe1[:], mybir.ActivationFunctionType.Ln)
        sph = consts.tile([Z, GPC], F32, name="sph")
        nc.vector.tensor_scalar_mul(sph[:], spt[:], 0.5)
        spb1h = consts.tile([Z, GPC], F32, name="spb1h")
        nc.vector.tensor_mul(spb1h[:], sph[:], b1t[:])

        swishes = {}  # g -> tile
        expos = {}  # g -> tile
        esums = {}  # q -> [C, 4] tile
        n_x = len(cfg["x_engines"])

        def stage1(q):
            """x/w loads, W1 matmuls, swish for quad q."""
            wt = wpool.tile([P, 4 * NCH * Z], F32R, tag="wt", name=f"wt{q}")
            we.dma_start(
                wt[:].rearrange("p (j c z) -> p j c z", j=4, c=NCH), w1m[q]
            )
            for j in range(4):
                g = 4 * q + j
                xe = _eng(nc, cfg["x_engines"][g % n_x])
                xt = xpool.tile([P, NCH * L], F32R, tag="xt", name=f"xt{g}")
                if cfg["x_split"]:
                    nc.sync.dma_start(
                        xt[:, : 3 * L].rearrange("p (c l) -> p c l", c=3),
                        xg[g, : 3 * P].rearrange("(c p) l -> p c l", p=P),
                    )
                    nc.scalar.dma_start(
                        xt[:, 3 * L : 6 * L].rearrange("p (c l) -> p c l", c=3),
                        xg[g, 3 * P : 6 * P].rearrange("(c p) l -> p c l", p=P),
                    )
                else:
                    xe.dma_start(
                        xt[:, : 6 * L].rearrange("p (c l) -> p c l", c=6),
                        xg[g, : 6 * P].rearrange("(c p) l -> p c l", p=P),
                    )
                xe.dma_start(xt[:KLAST, 6 * L :], xg[g, 6 * P :])

                h = hps.tile([Z, L], F32, tag="h", name=f"h{g}")
                for c in range(NCH):
                    kk = P if c < 6 else KLAST
                    nc.tensor.matmul(
                        h[:],
                        wt[:kk, (j * NCH + c) * Z : (j * NCH + c + 1) * Z],
                        xt[:kk, c * L : (c + 1) * L],
                        start=(c == 0),
                        stop=(c == NCH - 1),
                    )
                t = spool.tile([Z, L], F32, tag="t", name=f"t{g}")
                nc.scalar.activation(
                    t[:],
                    h[:],
                    mybir.ActivationFunctionType.Tanh,
                    bias=spb1h[:, g : g + 1],
                    scale=sph[:, g : g + 1],
                )
                u = spool.tile([Z, L], F32, tag="u", name=f"u{g}")
                nc.vector.tensor_scalar(
                    u[:],
                    h[:],
                    b1t[:, g : g + 1],
                    0.5,
                    op0=mybir.AluOpType.add,
                    op1=mybir.AluOpType.mult,
                )
                sw = dpool.tile([Z, L], F32R, tag="swish", name=f"sw{g}")
                nc.vector.scalar_tensor_tensor(
                    sw[:],
                    t[:],
                    1.0,
                    u[:],
                    op0=mybir.AluOpType.add,
                    op1=mybir.AluOpType.mult,
                )
                swishes[g] = sw

        def stage2(q):
            """W2 matmuls + exp for quad q (emitted one quad later)."""
            esum4 = e4pool.tile([C, 4], F32, tag="esum4", name=f"es4_{q}")
            esums[q] = esum4
            for j in range(4):
                g = 4 * q + j
                o = ops.tile([C, L], F32, tag="o", name=f"o{g}")
                nc.tensor.matmul(
                    o[:],
                    w2tt[:, g * C : (g + 1) * C],
                    swishes.pop(g)[:],
                    start=True,
                    stop=True,
                )
                expo = dpool.tile([C, L], F32, tag="expo", name=f"e{g}")
                nc.scalar.activation(
                    expo[:],
                    o[:],
                    mybir.ActivationFunctionType.Exp,
                    bias=b2t[:, g : g + 1],
                    scale=1.0,
                    accum_out=esum4[:, j : j + 1],
                )
                expos[g] = expo

        def stage3(q):
            """Normalization + store for quad q (emitted two quads later)."""
            esum4 = esums.pop(q)
            tot4 = tps.tile([1, 4], F32, tag="tb", name=f"tot{q}")
            nc.tensor.matmul(tot4[:], ones_k[:], esum4[:], start=True, stop=True)
            inv4 = spool.tile([1, 4], F32, tag="inv", name=f"inv{q}")
            nc.vector.reciprocal(inv4[:], tot4[:])
            bc4 = tps.tile([C, 4], F32, tag="tb", name=f"bc{q}")
            nc.tensor.matmul(bc4[:], ones_m[:], inv4[:], start=True, stop=True)
            invc4 = spool.tile([C, 4], F32, tag="invc", name=f"ic{q}")
            nc.vector.tensor_copy(invc4[:], bc4[:])
            for j in range(4):
                g = 4 * q + j
                res = spool.tile([C, L], F32, tag="res", name=f"r{g}")
                nc.vector.tensor_scalar_mul(
                    res[:], expos.pop(g)[:], invc4[:, j : j + 1]
                )
                oe.dma_start(out[g], res[:])

        for q in range(NQ):
            stage1(q)
            if q >= 1:
                stage2(q - 1)
            if q >= 2:
                stage3(q - 2)
        stage2(NQ - 1)
        stage3(NQ - 2)
        stage3(NQ - 1)

    nc.compile()
    return nc


def _marshal(x, W1, b1, beta, W2, b2, cfg=DEFAULT_CFG):
    """Full inputs -> list of per-core input dicts."""
    xg = np.ascontiguousarray(x, dtype=np.float32).reshape(B, X, L)
    w1T = W1.astype(np.float32, copy=False).transpose(0, 2, 1)  # [B, X, Z]
    w1m = np.zeros((B // 4, P, 4, NCH, Z), np.float32)
    if cfg["x_layout"] == "interleave":
        # w1m[gq, p, j, c, z] = W1T[4gq+j, 128c+p, z]
        main = w1T[:, : 6 * P].reshape(B // 4, 4, 6, P, Z)
        w1m[:, :, :, :6] = main.transpose(0, 3, 1, 2, 4)
        left = w1T[:, 6 * P :].reshape(B // 4, 4, KLAST, Z)
        w1m[:, :KLAST, :, 6] = left.transpose(0, 2, 1, 3)
    else:
        # row(p, c) = 7p+c for p<16, 112+6(p-16)+c for p>=16
        lo = w1T[:, : 7 * 16].reshape(B // 4, 4, 16, NCH, Z)
        hi = w1T[:, 7 * 16 :].reshape(B // 4, 4, 112, 6, Z)
        w1m[:, :16] = lo.transpose(0, 2, 1, 3, 4)
        w1m[:, 16:, :, :6] = hi.transpose(0, 2, 1, 3, 4)
    w2s = (W2.astype(np.float32, copy=False) * np.float32(1.0 / 1.1)).transpose(
        0, 2, 1
    )  # [B, Z, C]

    in_maps = []
    for core in range(NCORE):
        s = slice(core * GPC, (core + 1) * GPC)
        sq = slice(core * GPC // 4, (core + 1) * GPC // 4)
        in_maps.append(
            {
                "xg": xg[s],
                "w1m": w1m[sq],
                # [Z, GPC*C]: w2t[z, g*C+c] = W2[g0+g, c, z] / 1.1
                "w2t": np.ascontiguousarray(
                    w2s[s].transpose(1, 0, 2).reshape(Z, GPC * C)
                ),
                "b1c": np.ascontiguousarray(b1[s].astype(np.float32).T),
                "btc": np.ascontiguousarray(
                    np.broadcast_to(beta[s].astype(np.float32), (Z, GPC))
                ),
                "b2c": np.ascontiguousarray(b2[s].astype(np.float32).T),
            }
        )
    return in_maps


def _run(in_maps, cfg=DEFAULT_CFG, trace=False, tmpdir=None):
    key = str(sorted(cfg.items()))
    if key not in _CACHE:
        _CACHE[key] = _build(cfg)
    return run_bass_kernel_spmd(
        _CACHE[key],
        in_maps,
        core_ids=list(range(NCORE)),
        trace=trace,
        tmpdir=tmpdir,
    )


_LAST = {}


def kernel(x, W1, b1, beta, W2, b2):
    in_maps = _marshal(x, W1, b1, beta, W2, b2)
    trace = bool(os.environ.get("KERNEL_TRACE"))
    r = _run(in_maps, trace=trace, tmpdir=os.environ.get("KERNEL_TRACE_DIR"))
    _LAST["results"] = r
    outs = [r.results[c]["out"].reshape(GPC, C * L) for c in range(NCORE)]
    return np.concatenate(outs, axis=0)



# revision 6
# speedup vs baseline: 3.6353x; 3.6353x over previous
"""Grouped per-sample MLP (conv1d groups=B) + GroupSwish + softmax, on 8 NeuronCores.

Data-parallel over the group axis B=256: 32 groups per core, processed in
8 quads of 4 groups. Per group g: h = W1[g] @ x[g] + b1[g]; GroupSwish;
o = W2[g] @ h + b2[g]; softmax over the flattened [C*L] logits.

Key design points (vs the fp32 baseline at ~312us):
  - x is shipped as fp8e4m3 (12.8 MB/core), W1/W2 as bf16. End-to-end
    rel_fro error ~6.5e-3 (numpy-simulated), well under the 2e-2 gate.
  - Contraction X=784 is split 7x112 (not 6x128+16) so every matmul is
    K<=128, M=32: uniform (128,32) PE tile mode -> no mode-switch drains.
  - 4 groups share the 128-wide PE array via column tiling: matmuls for
    the 4 groups of a quad write PSUM partition strips 32j..32j+31 and
    run concurrently (tile_position auto-derived from out.base_partition).
    Emission is chunk-outer / group-inner so the 4 strips stay busy.
  - Activations/DVE ops run on whole [128, 512] quads (DVE/ACT cost is
    per-free-dim-element, not per-partition, so 4 groups cost 1 group).
  - W2 per group is embedded as a [128, 32] column block (rows 32j..+31
    hold W2[g].T/1.1, rest zero) -> o quad in one (128,32)-mode pass;
    pad logit rows compute as exactly 0.
  - Softmax cross-partition sum + broadcast with two (128,32)-mode
    matmuls against constant selector matrices; garbage is never
    multiplied by 0 anywhere (no NaN paths).
  - x DMA per quad is split across both HWDGE rings (sync + scalar),
    one contiguous 7168B run per partition.
"""

import os
import numpy as np
import ml_dtypes
from contextlib import ExitStack

import concourse.mybir as mybir
import concourse.tile as tile
from concourse import bacc
from concourse.bass_utils import run_bass_kernel_spmd

B, X, Z, C, L = 256, 784, 32, 10, 512
NCORE = 8
GPC = B // NCORE  # 32 groups per core
NQ = GPC // 4  # 8 quads of 4 groups
NCH = 7  # contraction chunks
KP = X // NCH  # 112 rows per chunk
F32 = mybir.dt.float32
BF16 = mybir.dt.bfloat16
FP8 = mybir.dt.float8e4

NP_BF16 = ml_dtypes.bfloat16
NP_FP8 = ml_dtypes.float8_e4m3fn

DEFAULT_CFG = dict(
    x_bufs=4,
    w_bufs=4,
    s_bufs=3,
    h_bufs=3,
    o_bufs=2,
)

_CACHE: dict = {}


def _build(cfg=DEFAULT_CFG):
    nc = bacc.Bacc("TRN2", target_bir_lowering=False, debug=False)

    xm = nc.dram_tensor("xm", [NQ, KP, 4 * NCH * L], FP8, kind="ExternalInput").ap()
    w1m = nc.dram_tensor("w1m", [NQ, KP, 4 * NCH * Z], BF16, kind="ExternalInput").ap()
    w2c = nc.dram_tensor("w2c", [128, NQ * 4 * Z], BF16, kind="ExternalInput").ap()
    onest = nc.dram_tensor("onest", [128, 4 * Z], BF16, kind="ExternalInput").ap()
    sphq = nc.dram_tensor("sphq", [128, NQ], F32, kind="ExternalInput").ap()
    spbq = nc.dram_tensor("spbq", [128, NQ], F32, kind="ExternalInput").ap()
    b1q = nc.dram_tensor("b1q", [128, NQ], F32, kind="ExternalInput").ap()
    b2q = nc.dram_tensor("b2q", [128, NQ], F32, kind="ExternalInput").ap()
    out = nc.dram_tensor("out", [GPC, C, L], BF16, kind="ExternalOutput").ap()

    with tile.TileContext(nc) as tc, ExitStack() as ctx:
        consts = ctx.enter_context(tc.tile_pool(name="consts", bufs=1))
        xpool = ctx.enter_context(tc.tile_pool(name="x", bufs=cfg["x_bufs"]))
        wpool = ctx.enter_context(tc.tile_pool(name="w1", bufs=cfg["w_bufs"]))
        spool = ctx.enter_context(tc.tile_pool(name="act", bufs=cfg["s_bufs"]))
        hps = ctx.enter_context(tc.tile_pool(name="hps", bufs=cfg["h_bufs"], space="PSUM"))
        ops = ctx.enter_context(tc.tile_pool(name="ops", bufs=cfg["o_bufs"], space="PSUM"))
        tps = ctx.enter_context(tc.tile_pool(name="tps", bufs=2, space="PSUM"))

        w2t = consts.tile([128, NQ * 4 * Z], BF16, name="w2t")
        nc.gpsimd.dma_start(w2t[:], w2c)
        ot = consts.tile([128, 4 * Z], BF16, name="ot")
        nc.gpsimd.dma_start(ot[:], onest)
        spht = consts.tile([128, NQ], F32, name="spht")
        nc.gpsimd.dma_start(spht[:], sphq)
        spbt = consts.tile([128, NQ], F32, name="spbt")
        nc.gpsimd.dma_start(spbt[:], spbq)
        b1t = consts.tile([128, NQ], F32, name="b1t")
        nc.gpsimd.dma_start(b1t[:], b1q)
        b2t = consts.tile([128, NQ], F32, name="b2t")
        nc.gpsimd.dma_start(b2t[:], b2q)

        for q in range(NQ):
            # --- loads ---
            xt = xpool.tile([128, 4 * NCH * L], FP8, tag="xt", name=f"xt{q}")
            half = 2 * NCH * L
            nc.sync.dma_start(xt[:KP, :half], xm[q, :, :half])
            nc.scalar.dma_start(xt[:KP, half:], xm[q, :, half:])
            wt = wpool.tile([128, 4 * NCH * Z], BF16, tag="wt", name=f"wt{q}")
            nc.gpsimd.dma_start(wt[:KP, :], w1m[q])

            # --- h quad: 4 groups x [32, 512], col-tiled, chunk-outer ---
            h4 = hps.tile([128, L], F32, tag="h", name=f"h{q}")
            for c in range(NCH):
                for j in range(4):
                    s = (j * NCH + c)
                    nc.tensor.matmul(
                        h4[32 * j : 32 * j + 32, :],
                        wt[:KP, s * Z : (s + 1) * Z],
                        xt[:KP, s * L : (s + 1) * L],
                        start=(c == 0),
                        stop=(c == NCH - 1),
                        tile_position=(0, 32 * j),
                    )

            # --- GroupSwish: ((h+b1)*0.5) * (1 + tanh(sp/2*(h+b1))) ---
            t = spool.tile([128, L], BF16, tag="t", name=f"t{q}")
            nc.scalar.activation(
                t[:],
                h4[:],
                mybir.ActivationFunctionType.Tanh,
                bias=spbt[:, q : q + 1],
                scale=spht[:, q : q + 1],
            )
            u = spool.tile([128, L], BF16, tag="u", name=f"u{q}")
            nc.vector.tensor_scalar(
                u[:],
                h4[:],
                b1t[:, q : q + 1],
                0.5,
                op0=mybir.AluOpType.add,
                op1=mybir.AluOpType.mult,
            )
            sw = spool.tile([128, L], BF16, tag="sw", name=f"sw{q}")
            nc.vector.scalar_tensor_tensor(
                sw[:],
                t[:],
                1.0,
                u[:],
                op0=mybir.AluOpType.add,
                op1=mybir.AluOpType.mult,
            )

            # --- o quad: 4 groups x [32(10 used), 512] via zero-padded W2 blocks ---
            o4 = ops.tile([128, L], F32, tag="o", name=f"o{q}")
            for j in range(4):
                g = 4 * q + j
                nc.tensor.matmul(
                    o4[32 * j : 32 * j + 32, :],
                    w2t[:, g * Z : (g + 1) * Z],
                    sw[:],
                    start=True,
                    stop=True,
                    tile_position=(0, 32 * j),
                )

            # --- softmax ---
            expo = spool.tile([128, L], BF16, tag="expo", name=f"e{q}")
            esum = spool.tile([128, 1], F32, tag="esum", name=f"es{q}")
            nc.scalar.activation(
                expo[:],
                o4[:],
                mybir.ActivationFunctionType.Exp,
                bias=b2t[:, q : q + 1],
                scale=1.0,
                accum_out=esum[:],
            )
            esb = spool.tile([128, 1], BF16, tag="esb", name=f"eb{q}")
            nc.vector.tensor_copy(esb[:], esum[:])
            # totb[32j+c] = sum of esum over the 10 logit rows of group j
            totb = tps.tile([128, 1], F32, tag="tb", name=f"tot{q}")
            for j in range(4):
                nc.tensor.matmul(
                    totb[32 * j : 32 * j + 32, :],
                    ot[:, 32 * j : 32 * j + 32],
                    esb[:],
                    start=True,
                    stop=True,
                    tile_position=(0, 32 * j),
                )
            invb = spool.tile([128, 1], F32, tag="invb", name=f"iv{q}")
            nc.vector.reciprocal(invb[:], totb[:])
            res = spool.tile([128, L], BF16, tag="res", name=f"r{q}")
            nc.vector.tensor_scalar_mul(res[:], expo[:], invb[:])
            for j in range(4):
                nc.gpsimd.dma_start(out[4 * q + j], res[32 * j : 32 * j + 10, :])

    nc.compile()
    return nc


def _marshal(x, W1, b1, beta, W2, b2):
    """Full inputs -> list of per-core input dicts (all layouts hardcoded)."""
    # x: [1, B*X, L] -> [B, 7, 112, L] -> per-quad partition-major fp8
    xg = np.asarray(x, dtype=np.float32).reshape(B, NCH, KP, L)
    x8 = xg.astype(NP_FP8)
    # [B/4 quads, 4, NCH, KP, L] -> [quads, KP, 4, NCH, L]
    x8 = x8.reshape(B // 4, 4, NCH, KP, L).transpose(0, 3, 1, 2, 4)
    x8 = np.ascontiguousarray(x8).reshape(B // 4, KP, 4 * NCH * L)

    # W1: [B, Z, X] -> lhsT chunks [quads, KP, 4, NCH, Z] bf16
    w1T = np.asarray(W1, dtype=np.float32).transpose(0, 2, 1)  # [B, X, Z]
    w1c = w1T.reshape(B // 4, 4, NCH, KP, Z).transpose(0, 3, 1, 2, 4)
    w1c = np.ascontiguousarray(w1c).astype(NP_BF16).reshape(B // 4, KP, 4 * NCH * Z)

    # W2 blockdiag: w2c[32j+z, g*Z+c-block] = W2[g, c, z]/1.1 (per core below)
    w2s = (np.asarray(W2, dtype=np.float32) * np.float32(1.0 / 1.1)).transpose(0, 2, 1)  # [B, Z, C]

    onest = np.zeros((128, 4 * Z), dtype=NP_BF16)
    for j in range(4):
        onest[32 * j : 32 * j + C, 32 * j : 32 * j + 32] = NP_BF16(1.0)

    b1f = np.asarray(b1, dtype=np.float32)
    b2f = np.asarray(b2, dtype=np.float32)
    spf = np.log1p(np.exp(np.asarray(beta, dtype=np.float64))).astype(np.float32)

    in_maps = []
    for core in range(NCORE):
        g0 = core * GPC
        sq = slice(core * NQ, (core + 1) * NQ)

        w2core = np.zeros((128, NQ * 4 * Z), dtype=np.float32)
        sph = np.zeros((128, NQ), dtype=np.float32)
        spb = np.zeros((128, NQ), dtype=np.float32)
        b1m = np.zeros((128, NQ), dtype=np.float32)
        b2m = np.zeros((128, NQ), dtype=np.float32)
        for q in range(NQ):
            for j in range(4):
                g = g0 + 4 * q + j
                w2core[32 * j : 32 * j + Z, (4 * q + j) * Z : (4 * q + j) * Z + C] = w2s[g]
                sph[32 * j : 32 * j + Z, q] = 0.5 * spf[g]
                spb[32 * j : 32 * j + Z, q] = 0.5 * spf[g] * b1f[g]
                b1m[32 * j : 32 * j + Z, q] = b1f[g]
                b2m[32 * j : 32 * j + C, q] = b2f[g]

        in_maps.append(
            {
                "xm": x8[sq],
                "w1m": w1c[sq],
                "w2c": w2core.astype(NP_BF16),
                "onest": onest,
                "sphq": sph,
                "spbq": spb,
                "b1q": b1m,
                "b2q": b2m,
            }
        )
    return in_maps


def _run(in_maps, cfg=DEFAULT_CFG, trace=False, tmpdir=None):
    key = str(sorted(cfg.items()))
    if key not in _CACHE:
        _CACHE[key] = _build(cfg)
    return run_bass_kernel_spmd(
        _CACHE[key],
        in_maps,
        core_ids=list(range(NCORE)),
        trace=trace,
        tmpdir=tmpdir,
    )


_LAST = {}


def kernel(x, W1, b1, beta, W2, b2):
    in_maps = _marshal(x, W1, b1, beta, W2, b2)
    trace = bool(os.environ.get("KERNEL_TRACE"))
    r = _run(in_maps, trace=trace, tmpdir=os.environ.get("KERNEL_TRACE_DIR"))
    _LAST["results"] = r
    outs = [
        r.results[c]["out"].astype(np.float32).reshape(GPC, C * L)
        for c in range(NCORE)
    ]
    return np.concatenate(outs, axis=0)
